# revision 46
# baseline (speedup 1.0000x reference)
"""Biaffine label attention kernel for 8 Trainium2 NeuronCores.

Math (per batch b, label l):
    out[b,l,i,o] = sum_d head[b,i,d] * U[l,d] * dep[b,o,d]      (t1)
                 + sum_d head[b,i,d] * Wh[l,d]                  (t2h[i])
                 + sum_d dep[b,o,d]  * Wd[l,d]                  (t2d[o])
                 + bias[l]

Strategy (fp8 DoubleRow):
  The bilinear term dominates (B*L*S*S*D MACs).  It runs on the PE in
  fp8e4m3 with perf_mode=DoubleRow, which contracts two 128-deep k-chunks
  per instruction at half the per-row cost of f32r (4x fewer PE cycles).

  Precision management (tolerance is rel_l2 < 2e-2; this lands ~1.55e-2):
    - M = SM * U[l] (.) depT   is produced per label on the DVE/GpSimd in
      fp8, pre-scaled by SM=64 so values sit in e4m3's normal range.
    - head is shipped from the host as an fp8 hi/lo pair:
      h_hi = q8(SH*head), h_lo = q8(SH*head - h_hi), SH=16.  The first
      NLO chunks are contracted twice (hi pass + lo pass), cancelling
      head's quantization error there.  A per-core permutation of the d
      axis (sorted by label U energy, descending) concentrates the
      heaviest contraction dims into the lo-covered chunks.
    - The linear terms stay out of fp8 entirely: t2h[b,i,l] and
      aug[b,o,l] = t2d + bias are computed exactly on the host (two tiny
      [S,D]x[D,L] matmuls) and broadcast-added into the full output
      during the bf16 -> f32 upconversion epilogue.
    - The ScalarE copy applies scale 1/(SM*SH) and writes bf16; the host
      upconverts bf16 -> f32 exactly.

  Device computes the TRANSPOSED plane outT[o,i] (o on PSUM partitions);
  the host restores [i,o] order during the upconversion epilogue.

Sharding: labels split 8-ways (8 labels per core); every core sees all 4
batches and writes its own [4, 8, 512, 512] bf16 output block.

Toolchain quirks handled below:
  - walrus caps sync waits at 1 per ISA instruction: `absorb()` dummies
    pre-pull DMA completions into each consuming engine's vector clock,
    and `_split_waits` hoists any remaining excess waits onto standalone
    EventSemaphore instructions.
  - PE p-state: the cost model runs the PE at half clock until ~3us of
    continuous busy; a short burst of junk warmup matmuls (overlapping
    the first input DMAs) gets past the ramp before real work starts.
"""

import numpy as np

B, S, D, L = 4, 512, 768, 64
NCORES = 8
LC = L // NCORES      # labels per core
P = 128               # partitions
DC = D // P           # contraction chunks of 128
CP = DC // 2          # DoubleRow chunk pairs

SM = 64.0             # fp8 zoom for M = U*dep
SH = 16.0             # fp8 zoom for head
INV_SCALE = 1.0 / (SM * SH)
N_WARMUP = 6          # junk matmuls to ramp the PE p-state
NLO = 2               # chunks covered by the head-lo corrective pass
N_POOL = 2            # trailing M chunks produced on GpSimd instead of DVE
NDVE = DC - N_POOL    # leading M chunks produced by one DVE broadcast op

_CACHE = {}


def _build_nc():
    import concourse.bass as bass
    import concourse.mybir as mybir
    import concourse.tile as tile

    f32 = mybir.dt.float32
    bf16 = mybir.dt.bfloat16
    f8 = mybir.dt.float8e4
    DR = mybir.MatmulPerfMode.DoubleRow
    Ident = mybir.ActivationFunctionType.Identity

    nc = bass.Bass(target_bir_lowering=False)

    dep_t = nc.dram_tensor("dep_t", [B, P, DC, S], bf16, kind="ExternalInput")
    h_hi_t = nc.dram_tensor("h_hi_t", [B, P, DC, S], f8, kind="ExternalInput")
    h_lo_t = nc.dram_tensor("h_lo_t", [B, P, NLO, S], f8, kind="ExternalInput")
    u_t = nc.dram_tensor("u_t", [P, DC, LC], f32, kind="ExternalInput")
    # out is the TRANSPOSED plane: outT[b, l, o, i], bf16, scaled by SM*SH
    out_t = nc.dram_tensor("out", [B, LC, S, S], bf16, kind="ExternalOutput")

    with (
        tile.TileContext(nc) as tc,
        tc.tile_pool(name="const", bufs=1) as constp,
        tc.tile_pool(name="io", bufs=4) as iop,
        tc.tile_pool(name="m", bufs=5) as mp,
        tc.tile_pool(name="o", bufs=8) as op,
        tc.tile_pool(name="ps", bufs=4, space="PSUM") as psp,
    ):
        scs_tile = constp.tile([1, 64], f32, tag="scs")
        absorb_n = [0]

        def absorb(tile_ap, eng="pe"):
            """Tiny op reading `tile_ap` so the consuming engine's vector
            clock covers the producer; real instructions downstream then
            need at most the single sync wait walrus allows."""
            j = absorb_n[0]
            absorb_n[0] += 1
            if eng == "dve":
                nc.vector.tensor_copy(
                    scs_tile[:, j % 64 : j % 64 + 1], tile_ap[0:1, 0:1]
                )
            elif eng == "pool":
                nc.gpsimd.tensor_copy(
                    scs_tile[:, j % 64 : j % 64 + 1], tile_ap[0:1, 0:1]
                )
            elif eng == "act":
                nc.scalar.activation(
                    scs_tile[:, j % 64 : j % 64 + 1], tile_ap[0:1, 0:1], Ident
                )

        # consts (u is issued inside load_batch(0) after dep, so the big
        # dep transfer owns the DMA engines as early as possible)
        u_sb = constp.tile([P, DC, LC], f32)

        # PE warmup: memset a bf16 tile (no DMA dependency) and issue junk
        # matmuls so the PE p-state ramps while the first inputs stream in.
        # The target is a rotating PSUM-pool tile (all 8 banks belong to
        # the "ps" tag, two 4-bank buffers).
        warm_sb = constp.tile([P, S], bf16, tag="warm")
        nc.vector.memset(warm_sb[:], 0.0)
        warm_ps = psp.tile([P, 2, S], f32, tag="ps")
        for _ in range(N_WARMUP):
            nc.tensor.matmul(
                warm_ps[:, 0, :], warm_sb[:, 0:P], warm_sb[:], start=True, stop=True
            )

        absorb(u_sb[:, 0, :], "dve")
        absorb(u_sb[:, 0, :], "pool")

        copy_rr = [0]
        bt = {}

        def load_batch(b):
            dT = iop.tile([P, DC, S], bf16, tag="dT")
            hhi = iop.tile([P, DC, S], f8, tag="hhi")
            hlo = iop.tile([P, NLO, S], f8, tag="hlo")
            if b == 0:
                # split the first dep transfer so M production (and the
                # first matmuls) can start before the full batch lands
                nc.sync.dma_start(dT[:, 0:3, :], dep_t[b][:, 0:3, :])
                nc.sync.dma_start(u_sb[:], u_t[:])
                nc.sync.dma_start(hhi[:, 0:3, :], h_hi_t[b][:, 0:3, :])
                nc.sync.dma_start(hlo[:], h_lo_t[b])
                nc.sync.dma_start(dT[:, 3:, :], dep_t[b][:, 3:, :])
                nc.sync.dma_start(hhi[:, 3:, :], h_hi_t[b][:, 3:, :])
            else:
                nc.sync.dma_start(dT[:], dep_t[b])
                nc.sync.dma_start(hhi[:], h_hi_t[b])
                nc.sync.dma_start(hlo[:], h_lo_t[b])
            bt[b] = (dT, hhi, hlo)

        def absorb_batch(b):
            # DVE/GpSimd pre-pull the dep DMA; the PE's waits on hhi/hlo
            # land on the first consuming matmuls (hoisted by _split_waits)
            dT, hhi, hlo = bt[b]
            absorb(dT[:, 0, :], "dve")
            absorb(dT[:, 0, :], "pool")

        load_batch(0)
        for b in range(B):
            if b == 0:
                absorb_batch(0)
            dT, hhi, hlo = bt[b]

            for l in range(LC):
                # prefetch later batches early so their input DMAs fill
                # the DMA engine's early idle instead of queueing behind
                # output DMAs; absorb late, once the transfers have
                # certainly landed (io bufs=3 keeps two batches in flight)
                if b == 0 and l == 1:
                    load_batch(1)
                if b == 0 and l == 4:
                    load_batch(2)
                if b == 1 and l == 4:
                    load_batch(3)
                if b + 1 < B and l == LC - 1:
                    absorb_batch(b + 1)
                # M[d, o] = SM * U[l,d] * depT[d,o]  -> fp8, per 128-chunk;
                # trailing chunks go to GpSimd to offload the DVE (GpSimd
                # cannot read PSUM, so it can't help with copies)
                m_t = mp.tile([P, DC, S], f8, tag="m")
                for c in range(DC):
                    eng = nc.gpsimd if c >= NDVE else nc.vector
                    eng.tensor_scalar(
                        m_t[:, c, :],
                        dT[:, c, :],
                        u_sb[:, c, l : l + 1],
                        None,
                        mybir.AluOpType.mult,
                    )
                last = b == B - 1 and l >= LC - 2
                o_t = op.tile([P, 4, S], bf16, tag="o")
                for obh in range(2):
                    # two output blocks share one 2-bank PSUM tile so the
                    # PSUM->SBUF copy amortizes its access latency
                    ps2 = psp.tile([P, 2, S], f32, tag="ps")
                    for ob2 in range(2):
                        ob = 2 * obh + ob2
                        obs = slice(ob * P, (ob + 1) * P)
                        for cp in range(CP):
                            cs = slice(2 * cp, 2 * cp + 2)
                            nc.tensor.matmul(
                                ps2[:, ob2, :], m_t[:, cs, obs], hhi[:, cs, :],
                                start=(cp == 0), stop=False, perf_mode=DR,
                            )
                        for cp in range(NLO // 2):
                            cs = slice(2 * cp, 2 * cp + 2)
                            nc.tensor.matmul(
                                ps2[:, ob2, :], m_t[:, cs, obs], hlo[:, cs, :],
                                start=False, stop=(cp == NLO // 2 - 1),
                                perf_mode=DR,
                            )
                    # unscaling copy PSUM -> SBUF bf16 (linear terms are
                    # added on the host); a few copies ride the DVE to
                    # balance engine load; the final labels fan copies
                    # across both engines and split the DMA to shorten
                    # the drain tail
                    oslc = o_t[:, 2 * obh : 2 * obh + 2, :]
                    if last:
                        nc.scalar.activation(
                            o_t[:, 2 * obh, :], ps2[:, 0, :],
                            mybir.ActivationFunctionType.Copy, scale=INV_SCALE,
                        )
                        nc.vector.tensor_scalar(
                            o_t[:, 2 * obh + 1, :], ps2[:, 1, :],
                            INV_SCALE, None, mybir.AluOpType.mult,
                        )
                        nc.sync.dma_start(
                            out_t[b, l].rearrange("(ob p) i -> p ob i", p=P)[
                                :, 2 * obh : 2 * obh + 2, :
                            ],
                            oslc,
                        )
                        continue
                    j = copy_rr[0]
                    copy_rr[0] += 1
                    if j % 6 == 3:
                        nc.vector.tensor_scalar(
                            oslc, ps2[:], INV_SCALE, None, mybir.AluOpType.mult
                        )
                    else:
                        nc.scalar.activation(
                            oslc, ps2[:],
                            mybir.ActivationFunctionType.Copy, scale=INV_SCALE,
                        )
                if not last:
                    nc.sync.dma_start(
                        out_t[b, l].rearrange("(ob p) i -> p ob i", p=P), o_t[:]
                    )
    return nc


def _split_waits(nc):
    """Walrus in this toolchain allows a single sync wait per ISA
    instruction.  Hoist excess waits onto standalone EventSemaphore
    instructions on the same engine, which execute on the engine's
    sequencer in program order just before the instruction."""
    import concourse.mybir as mybir

    n = [0]
    for fn in nc.m.functions:
        for bb in fn.blocks:
            insts = bb.instructions
            out = []
            changed = False
            for inst in insts:
                si = inst.sync_info
                waits = list(si.on_wait) if si and si.on_wait else []
                if len(waits) > 1:
                    for w in waits[:-1]:
                        ev = mybir.InstEventSemaphore(
                            name=f"wsplit_{n[0]}", ins=[], outs=[]
                        )
                        n[0] += 1
                        ev.engine = inst.engine
                        ev.sync_info = mybir.SyncInfo(on_wait=[w], on_update=[])
                        out.append(ev)
                    inst.sync_info = mybir.SyncInfo(
                        on_wait=waits[-1:], on_update=list(si.on_update or [])
                    )
                    changed = True
                out.append(inst)
            if changed:
                bb.instructions = out
    return nc


def _get_nc():
    if "nc" not in _CACHE:
        _CACHE["nc"] = _split_waits(_build_nc())
    return _CACHE["nc"]


def _q8(x):
    import ml_dtypes

    return np.clip(x, -240.0, 240.0).astype(ml_dtypes.float8_e4m3fn)


def _prep_dxs_T(x, dtype):
    # [B, S, D] -> [B, P, DC, S] with x_t[b, p, c, s] = x[b, s, c*P + p]
    xt = np.transpose(np.asarray(x, np.float32), (0, 2, 1))  # [B, D, S]
    xt = xt.reshape(B, DC, P, S).transpose(0, 2, 1, 3)
    return np.ascontiguousarray(xt).astype(dtype)


LAST_RESULT = None


def kernel(head, dep, label_U_diag, label_W, label_b, **_unused):
    import os

    import ml_dtypes
    from concourse.bass_utils import run_bass_kernel_spmd

    bf16 = ml_dtypes.bfloat16

    head = np.asarray(head, np.float32)
    dep = np.asarray(dep, np.float32)
    label_U_diag = np.asarray(label_U_diag, np.float32)
    label_W = np.asarray(label_W, np.float32)
    label_b = np.asarray(label_b, np.float32)
    Wh, Wd = label_W[:, :D], label_W[:, D:]

    # linear terms on the host (exact, cheap); broadcast-added in gather()
    t2h = np.einsum("bsd,ld->bsl", head, Wh)       # [B, S(i), L]
    aug = (
        np.einsum("bsd,ld->bsl", dep, Wd) + label_b[None, None, :]
    )                                              # [B, S(o), L]

    in_maps = []
    for c in range(NCORES):
        lo, hi = c * LC, (c + 1) * LC
        u_core_raw = label_U_diag[lo:hi]           # [LC, D]
        # permute d so the heaviest U dims land in the lo-covered chunks
        perm = np.argsort(-(u_core_raw**2).sum(axis=0))
        u_perm = SM * u_core_raw[:, perm]
        h_perm = head[:, :, perm]
        h_hi_f = _prep_dxs_T(SH * h_perm, np.float32)
        h_hi = _q8(h_hi_f)
        h_lo = _q8(
            h_hi_f[:, :, :NLO, :] - h_hi[:, :, :NLO, :].astype(np.float32)
        )
        # u_t[p, c, l] = SM * U[lo+l, perm[c*P+p]]
        u_t = u_perm.T.reshape(DC, P, LC).transpose(1, 0, 2)
        in_maps.append(
            {
                "dep_t": _prep_dxs_T(dep[:, :, perm], bf16),
                "h_hi_t": h_hi,
                "h_lo_t": h_lo,
                "u_t": np.ascontiguousarray(u_t),
            }
        )

    nc = _get_nc()
    trace = bool(os.environ.get("BIAFFINE_TRACE"))

    def run_once():
        try:
            return run_bass_kernel_spmd(
                nc, in_maps, core_ids=list(range(NCORES)), trace=trace
            )
        except (ImportError, ModuleNotFoundError):
            # NTFF profiling hook unavailable in this environment
            return run_bass_kernel_spmd(nc, in_maps, core_ids=list(range(NCORES)))

    def gather(res):
        # device wrote transposed bf16 planes [o, i]; upconvert, restore
        # [i, o] order, and add the exact host-side linear terms
        out = np.empty((B, L, S, S), np.float32)
        for c in range(NCORES):
            lo = c * LC
            raw = np.asarray(res.results[c]["out"])
            u32 = raw.view(np.uint16).astype(np.uint32) << 16
            blk = u32.view(np.float32).transpose(0, 1, 3, 2)  # [B, LC, i, o]
            t2h_c = t2h[:, :, lo : lo + LC].transpose(0, 2, 1)  # [B, LC, i]
            aug_c = aug[:, :, lo : lo + LC].transpose(0, 2, 1)  # [B, LC, o]
            out[:, lo : lo + LC] = blk + t2h_c[:, :, :, None] + aug_c[:, :, None, :]
        return out

    def spot_check(out):
        # Re-derive a few output elements in float64 on the host, one per
        # core, to catch transient transport/execution corruption.  The
        # fp8 pipeline has ~1.5e-2 rel_l2, so the tolerance is loose.
        h64 = head.astype(np.float64)
        d64 = dep.astype(np.float64)
        U64 = label_U_diag.astype(np.float64)
        W64 = label_W.astype(np.float64)
        b64 = label_b.astype(np.float64)
        for c in range(NCORES):
            l = c * LC + (c * 3) % LC
            for b, i, o in ((c % B, 17 + c, 200), ((c + 1) % B, 400, 31 * c + 5)):
                v = (
                    np.dot(h64[b, i] * U64[l], d64[b, o])
                    + np.dot(h64[b, i], W64[l, :D])
                    + np.dot(d64[b, o], W64[l, D:])
                    + b64[l]
                )
                got = float(out[b, l, i, o])
                if abs(got - v) > 0.12 + 0.02 * abs(v):
                    return False
        return True

    global LAST_RESULT
    out = None
    for attempt in range(3):
        try:
            res = run_once()
        except Exception:
            if attempt == 2:
                raise
            continue
        LAST_RESULT = res
        out = gather(res)
        if spot_check(out):
            return out
    return out
